# revision 9
# baseline (speedup 1.0000x reference)
"""HL1 ACE loss kernel for Trainium2, 8-core data-parallel over spatial.

Per core (per batch b): softmax over C=4 on the spatial shard, then the
three per-(b,c,bin) histogram families via cumulative thresholds:
  C_k = #{p_c >= t_k}          counts
  A_k = sum relu(p_c - t_k)    prob-mass above t_k  (=> per-bin sum_p)
  T_k = #{lab==c & p_c >= t_k} target counts
Custom DVE micro-ops pack TWO counts per pass into one f32 accumulator
(lo + 4096*hi; both fields <= 2048 so the sum stays integer-exact under
2^24), and fuse p-materialization with A_0 (MULSUM) and mask-build with
T_0 (MASKSUM). ACT carries exp, the relu (A) singles and a few sign (C)
singles. Host decodes the tiny [128, ncols] accumulators.
"""
import sys
sys.path.insert(0, "/opt/trn_rl_repo")
import os
import numpy as np

B, C = 4, 4
NBINS = 15
NCORES = 8
SP_FULL = 128 * 128 * 128          # spatial per (b,c), full problem
SP = SP_FULL // NCORES             # spatial per core = 262144
P, F = 128, SP // 128              # sbuf tile geometry 128 x 2048

EPS32 = np.float32(np.finfo(np.float32).eps)
BOUNDS = np.linspace(np.float32(0.0), np.float32(1.0) + EPS32, NBINS + 1,
                     dtype=np.float32)
TK = BOUNDS[1:]                    # t_1..t_15 (t_15 = 1+eps, never used)

PK = 4096.0                        # packing field multiplier

# ACT takes these C-thresholds as Sign singles; the rest pair up on DVE.
SIGN_C = [(0, 14), (1, 14), (2, 14), (3, 14),
          (0, 13), (1, 13), (2, 13), (3, 13)]


# ---- custom DVE op registration ------------------------------------------
def _register_ops():
    import concourse.dve_ops as dops
    from concourse.dve_spec import (Spec, Src0, Src1, C0, C1, C2, relu, eq,
                                    lower, _has_src1)
    from concourse.dve_uop import DveOpSpec
    from operator import add as _add

    def reg(name, body, accum=None, reference=None):
        for o in dops.OPS:
            if o.name == name:
                return o
        row = dops._CUSTOM_DVE_ROW_BASE + len(dops.OPS)
        spec = Spec(body=body, accum=accum, reference=reference)
        sha = {}
        for ver in ("v3", "v4"):
            u = lower(spec, ver=ver)
            sha[ver] = DveOpSpec(name=name, opcode=row, uops=u,
                                 rd1_en=_has_src1(spec)).sha(ver)
        op = dops.DveOp(name, spec, subdim=False, uops_sha=sha)
        dops.OPS.append(op)
        dops._SUB_OPCODE_FOR_NAME[name] = row
        dops.CUSTOM_DVE_SPECS[name] = spec
        return op

    cpack = reg("CPACK_K", (Src0 >= C0) + C2 * (Src0 >= C1), accum=_add,
                reference=lambda in0, s0, s1, imm2:
                (in0 >= s0) + imm2 * (in0 >= s1))
    tpack = reg("TPACK_K", ((Src0 >= C0) + C2 * (Src0 >= C1)) * Src1,
                accum=_add,
                reference=lambda in0, in1, s0, s1, imm2:
                ((in0 >= s0) + imm2 * (in0 >= s1)) * in1)
    mulsum = reg("MULSUM_K", Src0 * Src1, accum=_add,
                 reference=lambda in0, in1, s0, s1, imm2: in0 * in1)
    masksum = reg("MASKSUM_K", eq(Src0, C0), accum=_add,
                  reference=lambda in0, s0, s1, imm2:
                  (in0 == s0).astype(np.float32))
    return cpack, tpack, mulsum, masksum


def _build(nc, mybir):
    """Emit the SPMD program. Returns (nc, dve_cols, act_cols)."""
    CPACK, TPACK, MULSUM, MASKSUM = _register_ops()
    f32 = mybir.dt.float32
    AF = mybir.ActivationFunctionType
    AL = mybir.AluOpType

    lg = nc.dram_tensor("lg", [B, C, P, F], f32, kind="ExternalInput")
    lb = nc.dram_tensor("lb", [B, P, F], f32, kind="ExternalInput")

    # ---- column bookkeeping ------------------------------------------
    # DVE: ("A0",b,c) | ("T0",b,c) | ("CC",b,c,klo,khi) | ("TP",b,c,klo,khi)
    # ACT: ("A",b,c,k) k=1..14 | ("CS",b,c,k) for SIGN_C
    dve_cols, act_cols = [], []
    sign_c = {}
    for (c, k) in SIGN_C:
        sign_c.setdefault(c, set()).add(k)
    for b in range(B):
        for c in range(C):
            dve_cols.append(("A0", b, c))
            dve_cols.append(("T0", b, c))
            cks = [k for k in range(1, 15) if k not in sign_c.get(c, ())]
            if len(cks) % 2:
                cks.append(cks[-1])
            for i in range(0, len(cks), 2):
                dve_cols.append(("CC", b, c, cks[i], cks[i + 1]))
            tks = list(range(1, 15)) + [14]    # 14 thr -> 7 pairs (pad dup)
            for i in range(0, 14, 2):
                dve_cols.append(("TP", b, c, tks[i], tks[i + 1]))
            for k in range(1, 15):
                act_cols.append(("A", b, c, k))
            for k in sorted(sign_c.get(c, ())):
                act_cols.append(("CS", b, c, k))
    dmap = {it: i for i, it in enumerate(dve_cols)}
    amap = {it: i for i, it in enumerate(act_cols)}

    outV = nc.dram_tensor("outV", [P, len(dve_cols)], f32,
                          kind="ExternalOutput")
    outA = nc.dram_tensor("outA", [P, len(act_cols)], f32,
                          kind="ExternalOutput")

    # ---- const bias APs for ACT --------------------------------------
    bias_vals = {0.0}
    for k in range(1, 15):
        bias_vals.add(-float(TK[k - 1]))
    for v in sorted(bias_vals):
        t = nc.alloc_sbuf_tensor(
            f"cb_{abs(v):.7f}".replace(".", "_") + ("m" if v < 0 else "p"),
            [P, 1], f32)
        nc.gpsimd.memset(t.ap(), v)
        nc.const_aps.aps[(f32, v)] = t.ap()
    nc.all_engine_barrier()

    # ---- sbuf tiles ---------------------------------------------------
    def sb(name, shape, dt=f32):
        return nc.alloc_sbuf_tensor(name, shape, dt).ap()

    lgs = [sb(f"lgs{i}", [P, C * F]) for i in range(2)]   # logits -> e (exp)
    lbs = sb("lbs", [P, F])                               # labels (f32)
    ps = [sb(f"ps{i}", [P, C * F]) for i in range(2)]     # softmax probs
    S = sb("S", [P, F])
    R = sb("R", [P, F])
    rscr = sb("rscr", [P, F])
    m = sb("m", [P, F])                                    # per-class mask
    scr = sb("scr", [P, F])                                # packed-op out
    ascr = sb("ascr", [P, F])                              # ACT singles out
    accV = sb("accV", [P, len(dve_cols)])
    accA = sb("accA", [P, len(act_cols)])

    def pview(buf, c):
        return buf[:, c * F:(c + 1) * F]

    with (
        nc.Block() as block,
        nc.semaphore("dma_sem") as dma_sem,
        nc.semaphore("lg0_sem") as lg0_sem,
        nc.semaphore("lg1_sem") as lg1_sem,
        nc.semaphore("lg2_sem") as lg2_sem,
        nc.semaphore("lg3_sem") as lg3_sem,
        nc.semaphore("lb_sem") as lb_sem,
        nc.semaphore("ae_sem") as ae_sem,      # ACT exp(b) done: b+1
        nc.semaphore("as_sem") as as_sem,      # ACT singles(b) done: b+1
        nc.semaphore("vp_sem") as vp_sem,      # DVE p(b) ready: b+1
        nc.semaphore("vd_sem") as vd_sem,      # DVE packed(b) done: b+1
    ):
        lgc = [lg0_sem, lg1_sem, lg2_sem, lg3_sem]

        @block.sync
        def _(sync):
            for b in range(B):
                if b >= 2:
                    sync.wait_ge(vd_sem, b - 1)
                for c in range(C):
                    sync.dma_start(out=lgs[b % 2][:, c * F:(c + 1) * F],
                                   in_=lg[b, c]).then_inc(lgc[c], 16)
                if b >= 1:
                    sync.wait_ge(vd_sem, b)
                sync.dma_start(out=lbs, in_=lb[b]).then_inc(lb_sem, 16)
            sync.wait_ge(vd_sem, B)
            sync.wait_ge(as_sem, B)
            sync.dma_start(out=outV[:], in_=accV).then_inc(dma_sem, 16)
            sync.dma_start(out=outA[:], in_=accA).then_inc(dma_sem, 16)
            sync.wait_ge(lb_sem, 16 * B)
            sync.wait_ge(dma_sem, 32)

        @block.scalar
        def _(act):
            def exp(b):
                for c in range(C):
                    act.wait_ge(lgc[c], 16 * (b + 1))
                    ins = act.activation(out=pview(lgs[b % 2], c),
                                         in_=pview(lgs[b % 2], c), func=AF.Exp)
                    ins.then_inc(ae_sem, 1)

            def singles(b):
                pb = ps[b % 2]
                ins = None
                for cc in range(C):
                    act.wait_ge(vp_sem, 4 * b + cc + 1)
                    for (fam, bb, c, k) in act_cols:
                        if bb != b or c != cc:
                            continue
                        i0 = amap[(fam, bb, c, k)]
                        ins = act.activation(out=ascr, in_=pview(pb, c),
                                             func=AF.Relu if fam == "A"
                                             else AF.Sign,
                                             bias=-float(TK[k - 1]),
                                             accum_out=accA[:, i0:i0 + 1])
                ins.then_inc(as_sem, 1)

            exp(0)
            exp(1)
            singles(0)
            exp(2)
            singles(1)
            exp(3)
            singles(2)
            singles(3)

        @block.vector
        def _(vec):
            def prep(b):
                buf = b % 2
                e = lgs[buf]
                pb = ps[buf]
                vec.wait_ge(ae_sem, 4 * b + 2)
                vec.tensor_add(S, pview(e, 0), pview(e, 1))
                vec.wait_ge(ae_sem, 4 * b + 3)
                vec.tensor_add(S, S, pview(e, 2))
                vec.wait_ge(ae_sem, 4 * b + 4)
                vec.tensor_add(S, S, pview(e, 3))
                vec.reciprocal_approx_fast(out=R, in_=S)
                if b >= 2:
                    vec.wait_ge(as_sem, b - 1)
                for c in range(C):
                    ao = accV[:, dmap[("A0", b, c)]:dmap[("A0", b, c)] + 1]
                    vec._custom_dve(MULSUM, out=pview(pb, c),
                                    in0=pview(e, c), in1=R,
                                    accum_out=ao).then_inc(vp_sem, 1)

            def packed(b):
                pb = ps[b % 2]
                vec.wait_ge(lb_sem, 16 * (b + 1))
                ins = None
                for c in range(C):
                    ao = accV[:, dmap[("T0", b, c)]:dmap[("T0", b, c)] + 1]
                    vec._custom_dve(MASKSUM, out=m, in0=lbs,
                                    s0=float(c), accum_out=ao)
                    for it in dve_cols:
                        if it[0] == "TP" and it[1] == b and it[2] == c:
                            _, _, _, klo, khi = it
                            ao2 = accV[:, dmap[it]:dmap[it] + 1]
                            vec._custom_dve(
                                TPACK, out=scr, in0=pview(pb, c), in1=m,
                                s0=float(TK[klo - 1]), s1=float(TK[khi - 1]),
                                imm2=PK, accum_out=ao2)
                    for it in dve_cols:
                        if it[0] == "CC" and it[1] == b and it[2] == c:
                            _, _, _, klo, khi = it
                            ao2 = accV[:, dmap[it]:dmap[it] + 1]
                            ins = vec._custom_dve(
                                CPACK, out=scr, in0=pview(pb, c),
                                s0=float(TK[klo - 1]), s1=float(TK[khi - 1]),
                                imm2=PK, accum_out=ao2)
                ins.then_inc(vd_sem, 1)

            for b in range(B):
                prep(b)
                if b >= 1:
                    packed(b - 1)
            packed(B - 1)

    return nc, dve_cols, act_cols, dmap, amap


def _decode(dve_cols, act_cols, results):
    """Sum per-core [128, n] accumulators and decode into the cumulative
    family arrays Cf[b,c,k], Af[b,c,k], Tf[b,c,k] (k = 0..15)."""
    NV = len(dve_cols)
    totV = np.zeros(NV, np.float64)
    totA = np.zeros(len(act_cols), np.float64)
    # packed columns must be decoded per partition-row per core (fields are
    # only guaranteed <= 2048 per row), so split lo/hi before summing.
    lo_acc = np.zeros(NV, np.float64)
    hi_acc = np.zeros(NV, np.float64)
    for r in results:
        v = r["outV"].astype(np.float64)        # [128, NV]
        hi = np.floor(v / PK)
        lo = v - hi * PK
        lo_acc += lo.sum(0)
        hi_acc += hi.sum(0)
        totV += v.sum(0)
        totA += r["outA"].astype(np.float64).sum(0)

    Cf = np.zeros((B, C, 16))
    Af = np.zeros((B, C, 16))
    Tf = np.zeros((B, C, 16))
    Cf[:, :, 0] = SP_FULL
    n_cores = len(results)
    for i, it in enumerate(dve_cols):
        fam = it[0]
        if fam == "A0":
            Af[it[1], it[2], 0] = totV[i]
        elif fam == "T0":
            Tf[it[1], it[2], 0] = totV[i]
        elif fam == "CC":
            _, b, c, klo, khi = it
            Cf[b, c, klo] = lo_acc[i]
            Cf[b, c, khi] = hi_acc[i]
        else:  # TP
            _, b, c, klo, khi = it
            Tf[b, c, klo] = lo_acc[i]
            Tf[b, c, khi] = hi_acc[i]
    for i, it in enumerate(act_cols):
        fam, b, c, k = it
        if fam == "A":
            Af[b, c, k] = totA[i]
        else:  # CS: sign-encoded count
            Cf[b, c, k] = (totA[i] + SP_FULL) / 2.0
    return Cf, Af, Tf


def _finalize(Cf, Af, Tf):
    tk = np.zeros(16)
    tk[1:16] = TK.astype(np.float64)
    cnt = Cf[:, :, :15] - Cf[:, :, 1:16]
    S = Af[:, :, :15] + tk[:15] * Cf[:, :, :15]
    Sb = np.zeros((B, C, 15))
    Sb[:, :, :14] = S[:, :, :14] - S[:, :, 1:15]
    Sb[:, :, 14] = S[:, :, 14]
    tcb = Tf[:, :, :15] - Tf[:, :, 1:16]

    valid = cnt > 0.5
    denom = np.where(valid, cnt, 1.0)
    mean_p = Sb / denom
    mean_t = tcb / denom
    diff = np.where(valid, np.abs(mean_p - mean_t), 0.0)
    n_valid = np.maximum(valid.sum(-1), 1)
    ace = diff.sum(-1) / n_valid
    non_empty = (Tf[:, :, 0] > 0.5).astype(np.float64)
    return np.float32((ace * non_empty).mean())


def kernel(logits, labels):
    import concourse.bass as bass
    from concourse import mybir
    from concourse.bass_utils import run_bass_kernel_spmd

    nc = bass.Bass()
    nc, dve_cols, act_cols, dmap, amap = _build(nc, mybir)
    mybir.codegen_inst_isa_subclasses(nc)   # encode custom-DVE ISA bytes

    lgf = np.ascontiguousarray(np.asarray(logits).reshape(B, C, SP_FULL),
                               np.float32)
    lbl = np.asarray(labels).reshape(B, SP_FULL).astype(np.float32)

    in_maps = []
    for i in range(NCORES):
        sl = slice(i * SP, (i + 1) * SP)
        in_maps.append({
            "lg": np.ascontiguousarray(lgf[:, :, sl]).reshape(B, C, P, F),
            "lb": np.ascontiguousarray(lbl[:, sl]).reshape(B, P, F),
        })
    trace = bool(int(os.environ.get("KERNEL_TRACE", "0")))
    tmpdir = os.environ.get("KERNEL_TMPDIR") or None
    res = run_bass_kernel_spmd(nc, in_maps, list(range(NCORES)), trace=trace,
                               tmpdir=tmpdir)
    Cf, Af, Tf = _decode(dve_cols, act_cols, res.results)
    out = _finalize(Cf, Af, Tf)
    kernel._last = res
    return out


# revision 10
# speedup vs baseline: 1.0634x; 1.0634x over previous
"""HL1 ACE loss kernel for Trainium2, 8-core data-parallel over spatial.

Per core (per batch b): softmax over C=4 on the spatial shard, then the
three per-(b,c,bin) histogram families via cumulative thresholds:
  C_k = #{p_c >= t_k}          counts
  A_k = sum relu(p_c - t_k)    prob-mass above t_k  (=> per-bin sum_p)
  T_k = #{lab==c & p_c >= t_k} target counts
Custom DVE micro-ops pack TWO counts per pass into one f32 accumulator
(lo + 4096*hi; both fields <= 2048 so the sum stays integer-exact under
2^24), and fuse p-materialization with A_0 (MULSUM) and mask-build with
T_0 (MASKSUM). ACT carries exp, the relu (A) singles and a few sign (C)
singles. Host decodes the tiny [128, ncols] accumulators.
"""
import sys
sys.path.insert(0, "/opt/trn_rl_repo")
import os
import numpy as np

B, C = 4, 4
NBINS = 15
NCORES = 8
SP_FULL = 128 * 128 * 128          # spatial per (b,c), full problem
SP = SP_FULL // NCORES             # spatial per core = 262144
P, F = 128, SP // 128              # sbuf tile geometry 128 x 2048

EPS32 = np.float32(np.finfo(np.float32).eps)
BOUNDS = np.linspace(np.float32(0.0), np.float32(1.0) + EPS32, NBINS + 1,
                     dtype=np.float32)
TK = BOUNDS[1:]                    # t_1..t_15 (t_15 = 1+eps, never used)

PK = 4096.0                        # packing field multiplier

# ACT takes these C-thresholds as Sign singles; the rest pair up on DVE.
SIGN_C = [(0, 14), (1, 14), (2, 14), (3, 14),
          (0, 13), (1, 13), (2, 13), (3, 13)]


# ---- custom DVE op registration ------------------------------------------
def _register_ops():
    import concourse.dve_ops as dops
    from concourse.dve_spec import (Spec, Src0, Src1, C0, C1, C2, relu, eq,
                                    lower, _has_src1)
    from concourse.dve_uop import DveOpSpec
    from operator import add as _add

    def reg(name, body, accum=None, reference=None):
        for o in dops.OPS:
            if o.name == name:
                return o
        row = dops._CUSTOM_DVE_ROW_BASE + len(dops.OPS)
        spec = Spec(body=body, accum=accum, reference=reference)
        sha = {}
        for ver in ("v3", "v4"):
            u = lower(spec, ver=ver)
            sha[ver] = DveOpSpec(name=name, opcode=row, uops=u,
                                 rd1_en=_has_src1(spec)).sha(ver)
        op = dops.DveOp(name, spec, subdim=False, uops_sha=sha)
        dops.OPS.append(op)
        dops._SUB_OPCODE_FOR_NAME[name] = row
        dops.CUSTOM_DVE_SPECS[name] = spec
        return op

    cpack = reg("CPACK_K", (Src0 >= C0) + C2 * (Src0 >= C1), accum=_add,
                reference=lambda in0, s0, s1, imm2:
                (in0 >= s0) + imm2 * (in0 >= s1))
    tpack = reg("TPACK_K", ((Src0 >= C0) + C2 * (Src0 >= C1)) * Src1,
                accum=_add,
                reference=lambda in0, in1, s0, s1, imm2:
                ((in0 >= s0) + imm2 * (in0 >= s1)) * in1)
    mulsum = reg("MULSUM_K", Src0 * Src1, accum=_add,
                 reference=lambda in0, in1, s0, s1, imm2: in0 * in1)
    masksum = reg("MASKSUM_K", eq(Src0, C0), accum=_add,
                  reference=lambda in0, s0, s1, imm2:
                  (in0 == s0).astype(np.float32))
    return cpack, tpack, mulsum, masksum


def _build(nc, mybir):
    """Emit the SPMD program. Returns (nc, dve_cols, act_cols)."""
    CPACK, TPACK, MULSUM, MASKSUM = _register_ops()
    f32 = mybir.dt.float32
    AF = mybir.ActivationFunctionType
    AL = mybir.AluOpType

    lg = nc.dram_tensor("lg", [B, C, P, F], f32, kind="ExternalInput")
    lb = nc.dram_tensor("lb", [B, P, F], f32, kind="ExternalInput")

    # ---- column bookkeeping ------------------------------------------
    # DVE: ("A0",b,c) | ("T0",b,c) | ("CC",b,c,klo,khi) | ("TP",b,c,klo,khi)
    # ACT: ("A",b,c,k) k=1..14 | ("CS",b,c,k) for SIGN_C
    dve_cols, act_cols = [], []
    sign_c = {}
    for (c, k) in SIGN_C:
        sign_c.setdefault(c, set()).add(k)
    for b in range(B):
        for c in range(C):
            dve_cols.append(("A0", b, c))
            dve_cols.append(("T0", b, c))
            cks = [k for k in range(1, 15) if k not in sign_c.get(c, ())]
            if len(cks) % 2:
                cks.append(cks[-1])
            for i in range(0, len(cks), 2):
                dve_cols.append(("CC", b, c, cks[i], cks[i + 1]))
            tks = list(range(1, 15)) + [14]    # 14 thr -> 7 pairs (pad dup)
            for i in range(0, 14, 2):
                dve_cols.append(("TP", b, c, tks[i], tks[i + 1]))
            for k in range(1, 15):
                act_cols.append(("A", b, c, k))
            for k in sorted(sign_c.get(c, ())):
                act_cols.append(("CS", b, c, k))
    dmap = {it: i for i, it in enumerate(dve_cols)}
    amap = {it: i for i, it in enumerate(act_cols)}

    outV = nc.dram_tensor("outV", [P, len(dve_cols)], f32,
                          kind="ExternalOutput")
    outA = nc.dram_tensor("outA", [P, len(act_cols)], f32,
                          kind="ExternalOutput")

    # ---- const bias APs for ACT --------------------------------------
    bias_vals = {0.0}
    for k in range(1, 15):
        bias_vals.add(-float(TK[k - 1]))
    for v in sorted(bias_vals):
        t = nc.alloc_sbuf_tensor(
            f"cb_{abs(v):.7f}".replace(".", "_") + ("m" if v < 0 else "p"),
            [P, 1], f32)
        nc.gpsimd.memset(t.ap(), v)
        nc.const_aps.aps[(f32, v)] = t.ap()
    nc.all_engine_barrier()

    # ---- sbuf tiles ---------------------------------------------------
    def sb(name, shape, dt=f32):
        return nc.alloc_sbuf_tensor(name, shape, dt).ap()

    lgs = [sb(f"lgs{i}", [P, C * F]) for i in range(2)]   # logits -> e (exp)
    lbs = sb("lbs", [P, F])                               # labels (f32)
    ps = [sb(f"ps{i}", [P, C * F]) for i in range(2)]     # softmax probs
    S = sb("S", [P, F])
    R = sb("R", [P, F])
    rscr = sb("rscr", [P, F])
    m = sb("m", [P, F])                                    # per-class mask
    scr = sb("scr", [P, F])                                # packed-op out
    ascr = sb("ascr", [P, F])                              # ACT singles out
    accV = sb("accV", [P, len(dve_cols)])
    accA = sb("accA", [P, len(act_cols)])

    def pview(buf, c):
        return buf[:, c * F:(c + 1) * F]

    with (
        nc.Block() as block,
        nc.semaphore("dma_sem") as dma_sem,
        nc.semaphore("lg0_sem") as lg0_sem,
        nc.semaphore("lg1_sem") as lg1_sem,
        nc.semaphore("lg2_sem") as lg2_sem,
        nc.semaphore("lg3_sem") as lg3_sem,
        nc.semaphore("lb_sem") as lb_sem,
        nc.semaphore("ae_sem") as ae_sem,      # ACT exp(b) done: b+1
        nc.semaphore("as_sem") as as_sem,      # ACT singles(b) done: b+1
        nc.semaphore("vp_sem") as vp_sem,      # DVE p(b) ready: b+1
        nc.semaphore("vd_sem") as vd_sem,      # DVE packed(b) done: b+1
    ):
        lgc = [lg0_sem, lg1_sem, lg2_sem, lg3_sem]

        @block.sync
        def _(sync):
            for b in range(B):
                if b >= 2:
                    sync.wait_ge(vd_sem, b - 1)
                for c in range(C):
                    sync.dma_start(out=lgs[b % 2][:, c * F:(c + 1) * F],
                                   in_=lg[b, c]).then_inc(lgc[c], 16)
                if b >= 1:
                    sync.wait_ge(vd_sem, b)
                sync.dma_start(out=lbs, in_=lb[b]).then_inc(lb_sem, 16)
            sync.wait_ge(vd_sem, B)
            sync.wait_ge(as_sem, B)
            sync.dma_start(out=outV[:], in_=accV).then_inc(dma_sem, 16)
            sync.dma_start(out=outA[:], in_=accA).then_inc(dma_sem, 16)
            sync.wait_ge(lb_sem, 16 * B)
            sync.wait_ge(dma_sem, 32)

        @block.scalar
        def _(act):
            def exp(b):
                for c in range(C):
                    act.wait_ge(lgc[c], 16 * (b + 1))
                    ins = act.activation(out=pview(lgs[b % 2], c),
                                         in_=pview(lgs[b % 2], c), func=AF.Exp)
                    ins.then_inc(ae_sem, 1)

            def singles(b):
                pb = ps[b % 2]
                ins = None
                for cc in range(C):
                    act.wait_ge(vp_sem, 4 * b + cc + 1)
                    for (fam, bb, c, k) in act_cols:
                        if bb != b or c != cc:
                            continue
                        i0 = amap[(fam, bb, c, k)]
                        ins = act.activation(out=ascr, in_=pview(pb, c),
                                             func=AF.Relu if fam == "A"
                                             else AF.Sign,
                                             bias=-float(TK[k - 1]),
                                             accum_out=accA[:, i0:i0 + 1])
                ins.then_inc(as_sem, 1)

            exp(0)
            exp(1)
            singles(0)
            exp(2)
            singles(1)
            exp(3)
            singles(2)
            singles(3)

        @block.vector
        def _(vec):
            for b in range(B):
                buf = b % 2
                e = lgs[buf]
                pb = ps[buf]
                vec.wait_ge(ae_sem, 4 * b + 2)
                vec.tensor_add(S, pview(e, 0), pview(e, 1))
                vec.wait_ge(ae_sem, 4 * b + 3)
                vec.tensor_add(S, S, pview(e, 2))
                vec.wait_ge(ae_sem, 4 * b + 4)
                vec.tensor_add(S, S, pview(e, 3))
                vec.reciprocal_approx_fast(out=R, in_=S)
                if b >= 2:
                    vec.wait_ge(as_sem, b - 1)
                for c in range(C):
                    ao = accV[:, dmap[("A0", b, c)]:dmap[("A0", b, c)] + 1]
                    vec._custom_dve(MULSUM, out=pview(pb, c),
                                    in0=pview(e, c), in1=R,
                                    accum_out=ao).then_inc(vp_sem, 1)
                vec.wait_ge(lb_sem, 16 * (b + 1))
                for c in range(C):
                    ao = accV[:, dmap[("T0", b, c)]:dmap[("T0", b, c)] + 1]
                    vec._custom_dve(MASKSUM, out=m, in0=lbs,
                                    s0=float(c), accum_out=ao)
                    for it in dve_cols:
                        if it[0] == "TP" and it[1] == b and it[2] == c:
                            _, _, _, klo, khi = it
                            ao2 = accV[:, dmap[it]:dmap[it] + 1]
                            vec._custom_dve(
                                TPACK, out=scr, in0=pview(pb, c), in1=m,
                                s0=float(TK[klo - 1]), s1=float(TK[khi - 1]),
                                imm2=PK, accum_out=ao2)
                    for it in dve_cols:
                        if it[0] == "CC" and it[1] == b and it[2] == c:
                            _, _, _, klo, khi = it
                            ao2 = accV[:, dmap[it]:dmap[it] + 1]
                            ins = vec._custom_dve(
                                CPACK, out=scr, in0=pview(pb, c),
                                s0=float(TK[klo - 1]), s1=float(TK[khi - 1]),
                                imm2=PK, accum_out=ao2)
                ins.then_inc(vd_sem, 1)

    return nc, dve_cols, act_cols, dmap, amap


def _decode(dve_cols, act_cols, results):
    """Sum per-core [128, n] accumulators and decode into the cumulative
    family arrays Cf[b,c,k], Af[b,c,k], Tf[b,c,k] (k = 0..15)."""
    NV = len(dve_cols)
    totV = np.zeros(NV, np.float64)
    totA = np.zeros(len(act_cols), np.float64)
    # packed columns must be decoded per partition-row per core (fields are
    # only guaranteed <= 2048 per row), so split lo/hi before summing.
    lo_acc = np.zeros(NV, np.float64)
    hi_acc = np.zeros(NV, np.float64)
    for r in results:
        v = r["outV"].astype(np.float64)        # [128, NV]
        hi = np.floor(v / PK)
        lo = v - hi * PK
        lo_acc += lo.sum(0)
        hi_acc += hi.sum(0)
        totV += v.sum(0)
        totA += r["outA"].astype(np.float64).sum(0)

    Cf = np.zeros((B, C, 16))
    Af = np.zeros((B, C, 16))
    Tf = np.zeros((B, C, 16))
    Cf[:, :, 0] = SP_FULL
    n_cores = len(results)
    for i, it in enumerate(dve_cols):
        fam = it[0]
        if fam == "A0":
            Af[it[1], it[2], 0] = totV[i]
        elif fam == "T0":
            Tf[it[1], it[2], 0] = totV[i]
        elif fam == "CC":
            _, b, c, klo, khi = it
            Cf[b, c, klo] = lo_acc[i]
            Cf[b, c, khi] = hi_acc[i]
        else:  # TP
            _, b, c, klo, khi = it
            Tf[b, c, klo] = lo_acc[i]
            Tf[b, c, khi] = hi_acc[i]
    for i, it in enumerate(act_cols):
        fam, b, c, k = it
        if fam == "A":
            Af[b, c, k] = totA[i]
        else:  # CS: sign-encoded count
            Cf[b, c, k] = (totA[i] + SP_FULL) / 2.0
    return Cf, Af, Tf


def _finalize(Cf, Af, Tf):
    tk = np.zeros(16)
    tk[1:16] = TK.astype(np.float64)
    cnt = Cf[:, :, :15] - Cf[:, :, 1:16]
    S = Af[:, :, :15] + tk[:15] * Cf[:, :, :15]
    Sb = np.zeros((B, C, 15))
    Sb[:, :, :14] = S[:, :, :14] - S[:, :, 1:15]
    Sb[:, :, 14] = S[:, :, 14]
    tcb = Tf[:, :, :15] - Tf[:, :, 1:16]

    valid = cnt > 0.5
    denom = np.where(valid, cnt, 1.0)
    mean_p = Sb / denom
    mean_t = tcb / denom
    diff = np.where(valid, np.abs(mean_p - mean_t), 0.0)
    n_valid = np.maximum(valid.sum(-1), 1)
    ace = diff.sum(-1) / n_valid
    non_empty = (Tf[:, :, 0] > 0.5).astype(np.float64)
    return np.float32((ace * non_empty).mean())


def kernel(logits, labels):
    import concourse.bass as bass
    from concourse import mybir
    from concourse.bass_utils import run_bass_kernel_spmd

    nc = bass.Bass()
    nc, dve_cols, act_cols, dmap, amap = _build(nc, mybir)
    mybir.codegen_inst_isa_subclasses(nc)   # encode custom-DVE ISA bytes

    lgf = np.ascontiguousarray(np.asarray(logits).reshape(B, C, SP_FULL),
                               np.float32)
    lbl = np.asarray(labels).reshape(B, SP_FULL).astype(np.float32)

    in_maps = []
    for i in range(NCORES):
        sl = slice(i * SP, (i + 1) * SP)
        in_maps.append({
            "lg": np.ascontiguousarray(lgf[:, :, sl]).reshape(B, C, P, F),
            "lb": np.ascontiguousarray(lbl[:, sl]).reshape(B, P, F),
        })
    trace = bool(int(os.environ.get("KERNEL_TRACE", "0")))
    tmpdir = os.environ.get("KERNEL_TMPDIR") or None
    res = run_bass_kernel_spmd(nc, in_maps, list(range(NCORES)), trace=trace,
                               tmpdir=tmpdir)
    Cf, Af, Tf = _decode(dve_cols, act_cols, res.results)
    out = _finalize(Cf, Af, Tf)
    kernel._last = res
    return out
